# revision 7
# baseline (speedup 1.0000x reference)
"""Trainium2 Bass kernel for nn_Contraction — flipped orientation.

Per node b (one node = 128 channels = one partition-block):
  out1[c, (w,x2,v)] = sum_ki t4[(k,i), c] * U3cat[(k,i), (w,x2,v)]   (PE)
  out2[c, (w,x2)]   = sum_v out1[c, (w,x2,v)] * x[c, v]        (Pool mul + DVE X-reduce)
  c1[c, (w,x2)]     = u1w[(w,x2)] * wn1[c] + out2                    (Pool STT)
  out3[c, w]        = sum_x2 c1[c, (w,x2)] * x[c, x2]         (Pool mul + DVE X-reduce)

PE: 3 matmuls per node, lhsT = t4 chunk [<=128 k-rows, 128 c-cols] (streamed
weights), rhs = U3cat [k, 768] stationary in SBUF, out PSUM [128, 768] f32.
N-cycle cost = 3*768 per node = the dense-FLOP floor; no selector matmuls,
no sel9/o3 matmuls, no xrep DMA.

ACT copies PSUM out1 -> SBUF bf16 (GPSIMD cannot read PSUM); Pool does the
bf16 elementwise muls; DVE does the segmented X-axis add-reduces (f32 out).

End phase: outsb [c, (b, w)] f32 -> 3 transposes -> [b, (c, w)] -> one DMA.

Host prep: t4 = wn3*x products packed [3, 128, B, C] bf16 (chunk2 rows
112:117 carry wn2 for the folded U2 term); x as [c, b, i] bf16; wn1 as
[c, b] bf16; U3cat/u1wrep consts bf16.
"""

import sys

if "/opt/trn_rl_repo" not in sys.path:
    sys.path.insert(0, "/opt/trn_rl_repo")

import numpy as np
import ml_dtypes

import concourse.bass as bass
import concourse.mybir as mybir
import concourse.tile as tile
from concourse.masks import make_identity

dt = mybir.dt

B, C, ELL, EQ, E = 1024, 128, 16, 3, 10
P3, P2, P1 = 23, 5, 1
N_CORES = 8
BS = B // N_CORES          # nodes per core (128)
WXV = EQ * ELL * ELL       # 768
WX2 = EQ * ELL             # 48
KCH = (128, 128, 112 + P2) # K chunks (chunk2: 112 U3-rows + 5 U2-rows)
CH = 16                    # nodes per DMA chunk
NCH = BS // CH             # chunks per core (8)
FC = CH * C                # chunk free width (2048)

_f32 = dt.float32
_bf16 = dt.bfloat16
_bf = ml_dtypes.bfloat16
_mult = mybir.AluOpType.mult
_add = mybir.AluOpType.add
_AX = mybir.AxisListType.X


def _build_program():
    nc = bass.Bass("TRN2", target_bir_lowering=False, debug=False)

    trep_d = nc.dram_tensor("trep", [3, 128, BS, C], _bf16, kind="ExternalInput")
    xf_d = nc.dram_tensor("xf", [C, BS, ELL], _bf16, kind="ExternalInput")
    wn1_d = nc.dram_tensor("wn1", [C, BS], _bf16, kind="ExternalInput")
    u1w_d = nc.dram_tensor("u1w", [C, WX2], _bf16, kind="ExternalInput")
    u3cat_d = nc.dram_tensor("u3cat", [3, 128, WXV], _bf16, kind="ExternalInput")
    out_d = nc.dram_tensor("out", [BS, C * EQ], _f32, kind="ExternalOutput")

    with tile.TileContext(nc) as tc:
        with tc.tile_pool(name="const", bufs=1) as cpool:
            u3sb = cpool.tile([128, 3, WXV], _bf16)
            nc.sync.dma_start(out=u3sb[:], in_=u3cat_d[:].rearrange("j p f -> p j f"))
            u1sb = cpool.tile([C, WX2], _bf16)
            nc.sync.dma_start(out=u1sb[:], in_=u1w_d[:])
            xfsb = cpool.tile([C, BS, ELL], _bf16)
            nc.sync.dma_start(out=xfsb[:], in_=xf_d[:])
            wn1sb = cpool.tile([C, BS], _bf16)
            nc.sync.dma_start(out=wn1sb[:], in_=wn1_d[:])
            outsb = cpool.tile([C, BS * EQ], _f32)    # [c, (b, w)] staging

            with tc.tile_pool(name="work", bufs=2) as pool, \
                 tc.tile_pool(name="nwork", bufs=3) as npool, \
                 tc.tile_pool(name="ps1", bufs=3, space="PSUM") as psb:
                for ci in range(NCH):
                    bsl = slice(ci * CH, (ci + 1) * CH)
                    tch = pool.tile([128, 3, FC], _bf16, tag="tch")
                    nc.sync.dma_start(
                        out=tch[:], in_=trep_d[:, :, bsl].rearrange("j p b c -> p j b c")
                    )
                    for ni in range(CH):
                        n = ci * CH + ni
                        nsl = slice(ni * C, (ni + 1) * C)

                        HV = WXV // 2
                        o1b = npool.tile([C, WXV], _bf16, tag="o1b")
                        for h in range(2):
                            ps1 = psb.tile([C, HV], _f32, tag=f"ps1{h}")
                            for j in range(3):
                                nc.tensor.matmul(
                                    ps1[:],
                                    tch[: KCH[j], j, nsl],
                                    u3sb[: KCH[j], j, h * HV : (h + 1) * HV],
                                    start=(j == 0),
                                    stop=(j == 2),
                                )
                            nc.scalar.copy(o1b[:, h * HV : (h + 1) * HV], ps1[:])

                        xv = xfsb[:, n, None, :]
                        m1 = npool.tile([C, WX2, ELL], _bf16, tag="m1")
                        nc.gpsimd.tensor_mul(
                            m1[:],
                            o1b[:].rearrange("p (a v) -> p a v", v=ELL),
                            xv.to_broadcast([C, WX2, ELL]),
                        )
                        out2 = npool.tile([C, WX2], _f32, tag="out2")
                        nc.vector.tensor_reduce(out2[:], m1[:], _AX, _add)

                        # c1 = u1w * wn1[c,n] + out2
                        c1b = npool.tile([C, WX2], _bf16, tag="c1b")
                        nc.vector.scalar_tensor_tensor(
                            c1b[:], u1sb[:], wn1sb[:, n, None], out2[:], _mult, _add
                        )
                        m2 = npool.tile([C, EQ, ELL], _bf16, tag="m2")
                        nc.gpsimd.tensor_mul(
                            m2[:],
                            c1b[:].rearrange("p (w i) -> p w i", i=ELL),
                            xv.to_broadcast([C, EQ, ELL]),
                        )
                        nc.vector.tensor_reduce(
                            outsb[:, n * EQ : (n + 1) * EQ], m2[:], _AX, _add
                        )

            # ---------------- end phase: layout transform ----------------
            with tc.tile_pool(name="fin", bufs=2) as fpool, \
                 tc.tile_pool(name="ps_fin", bufs=2, space="PSUM") as psf:
                ident128 = cpool.tile([128, 128], _f32)
                make_identity(nc, ident128[:])

                finsb = fpool.tile([BS, C * EQ], _f32, tag="finsb")
                outsb_r = outsb[:].rearrange("c (b w) -> c b w", w=EQ)
                finsb_r = finsb[:].rearrange("b (c w) -> b c w", w=EQ)
                for w in range(EQ):
                    fin_ps = psf.tile([BS, C], _f32, tag="fin")
                    nc.tensor.transpose(fin_ps[:], outsb_r[:, :, w], ident128[:])
                    nc.scalar.copy(finsb_r[:, :, w], fin_ps[:])

                nc.sync.dma_start(out=out_d[:], in_=finsb[:])

    import bass_rust
    bass_rust.move_matmul_waits_to_ldweights(nc.m)
    bass_rust.generate_event_semaphores(nc)
    return nc


def _host_prep(x, y, U3, U2, U1, w_max, w2, w1):
    x = np.ascontiguousarray(x, dtype=np.float32)
    elem = np.argmax(y, axis=1)

    wn3 = w_max[elem]                       # [B, 23, C]
    wn1 = w1[elem][:, 0, :]                 # [B, C]

    # trep[j, p, b, c] = x[b, c, i(p)] * wn3[b, 8j + p//16, c]; chunk2 rows
    # 112:117 = wn2 (folded U2 contraction operand)
    trep = np.zeros((B, 3, 128, C), dtype=np.float32)
    wn3r = np.repeat(wn3, ELL, axis=1)      # [B, 368, C]
    xtile = np.tile(x.transpose(0, 2, 1), (1, P3, 1))  # [B, 368, C]
    trep.reshape(B, 384, C)[:, :368, :] = wn3r * xtile
    trep[:, 2, 112 : 112 + P2, :] = w2[elem]
    trep = np.ascontiguousarray(trep.transpose(1, 2, 0, 3)).astype(_bf)  # [3,128,B,C]

    xf = np.ascontiguousarray(x.transpose(1, 0, 2)).astype(_bf)   # [C, B, ELL]
    wn1c = np.ascontiguousarray(wn1.T).astype(_bf)                # [C, B]

    # U3cat: [(k,i), (w, x2, v)] chunks of 128; chunk2 rows 112:117 = U2
    u3k = U3.transpose(4, 3, 0, 1, 2).reshape(ELL * P3, WXV)
    u2k = U2.transpose(3, 0, 1, 2).reshape(P2, WXV)
    u3cat = np.zeros((3, 128, WXV), dtype=np.float32)
    u3cat[0] = u3k[0:128]
    u3cat[1] = u3k[128:256]
    u3cat[2, 0:112] = u3k[256:368]
    u3cat[2, 112 : 112 + P2] = u2k
    u3cat = u3cat.astype(_bf)

    u1wrep = np.tile(U1[:, :, 0].reshape(1, WX2), (C, 1)).astype(_bf)  # [C, 48]

    shared = {"u3cat": u3cat, "u1w": u1wrep}

    def per_core(ci):
        s = slice(ci * BS, (ci + 1) * BS)
        m = {
            "trep": np.ascontiguousarray(trep[:, :, s]),
            "xf": np.ascontiguousarray(xf[:, s]),
            "wn1": np.ascontiguousarray(wn1c[:, s]),
        }
        m.update(shared)
        return m

    return per_core


_PROGRAM_CACHE = {}


def kernel(**inputs) -> np.ndarray:
    from concourse.bass_utils import run_bass_kernel_spmd

    per_core = _host_prep(
        np.asarray(inputs["x"]), np.asarray(inputs["y"]),
        np.asarray(inputs["U3"]), np.asarray(inputs["U2"]),
        np.asarray(inputs["U1"]), np.asarray(inputs["w_max"]),
        np.asarray(inputs["w2"]), np.asarray(inputs["w1"]),
    )

    if "nc" not in _PROGRAM_CACHE:
        _PROGRAM_CACHE["nc"] = _build_program()
    nc = _PROGRAM_CACHE["nc"]

    in_maps = [per_core(ci) for ci in range(N_CORES)]
    res = run_bass_kernel_spmd(nc, in_maps, core_ids=list(range(N_CORES)))
    out = np.concatenate([r["out"] for r in res.results], axis=0)
    return out.astype(np.float32)


if __name__ == "__main__":
    from concourse.bass_interp import CoreSim

    rng = np.random.default_rng(0)
    x = rng.standard_normal((B, C, ELL)).astype(np.float32)
    elem = rng.integers(0, E, size=B)
    y = np.eye(E, dtype=np.float32)[elem]
    U3 = (rng.standard_normal((EQ, ELL, ELL, ELL, P3)) * 0.1).astype(np.float32)
    U2 = (rng.standard_normal((EQ, ELL, ELL, P2)) * 0.1).astype(np.float32)
    U1 = (rng.standard_normal((EQ, ELL, P1)) * 0.1).astype(np.float32)
    w_max = (rng.standard_normal((E, P3, C)) / P3).astype(np.float32)
    w2 = (rng.standard_normal((E, P2, C)) / P2).astype(np.float32)
    w1 = (rng.standard_normal((E, P1, C)) / P1).astype(np.float32)

    per_core = _host_prep(x, y, U3, U2, U1, w_max, w2, w1)
    nc = _build_program()
    sim = CoreSim(nc)
    m = per_core(0)
    for k, v in m.items():
        sim.tensor(k)[:] = v
    sim.simulate(check_with_hw=False, trace_hw=False)
    got = np.array(sim.tensor("out"))

    def ref_np(x, y, U3, U2, U1, w_max, w2, w1):
        wn3 = np.einsum("be,ekc->bkc", y, w_max)
        t = np.einsum("bkc,bci->bcik", wn3, x)
        out = np.einsum("wxvik,bcik->bcwxv", U3, t)
        wn2 = np.einsum("be,ekc->bkc", y, w2)
        c2 = np.einsum("wxvk,bkc->bcwxv", U2, wn2) + out
        out = np.einsum("bcwxi,bci->bcwx", c2, x)
        wn1 = np.einsum("be,ekc->bkc", y, w1)
        c1 = np.einsum("wxk,bkc->bcwx", U1, wn1) + out
        out = np.einsum("bcwi,bci->bcw", c1, x)
        return out.reshape(out.shape[0], -1)

    want = ref_np(x[:BS], y[:BS], U3, U2, U1, w_max, w2, w1)
    err = np.abs(got - want).max() / (np.abs(want).max() + 1e-30)
    print(f"CoreSim vs numpy rel err: {err:.3e}")
    assert err < 2e-2, "FAIL"
    print("SIM PASS")


# revision 8
# speedup vs baseline: 1.3225x; 1.3225x over previous
"""Trainium2 Bass kernel for nn_Contraction — flipped orientation, v2.

Per node b (one node = 128 channels = one partition-block):
  out1[c, (w,x2,v)] = sum_ki t4[(k,i), c] * U3cat[(k,i), (w,x2,v)]   (PE)
  out2[c, (w,x2)]   = sum_v out1[c, (w,x2,v)] * x[c, v]   (Pool mul + DVE X-reduce)
  out3[c, w]        = sum_x2 out2[c, (w,x2)] * x[c, x2]   (Pool mul + DVE X-reduce)
  final[b, c, w]    = out3 + q  where q = wn1*(U1 . x) is host-precomputed
                      and added during the end-phase transpose copies.

PE does ONLY the U3 contraction (6 matmuls/node, 2304 output-el-cycles =
dense-FLOP floor) plus 3 end transposes. No selector matmuls, no xrep.
The U2 term rides rows 112:117 of K-chunk 2; the U1 term is folded into
the end-phase PSUM->SBUF adds (q shipped from host).

Engine split: ACT copies PSUM out1 -> SBUF bf16 (one copy per node from a
padded 2-bank PSUM tile); Pool (GPSIMD) does the bf16 elementwise muls
(cannot read PSUM); DVE does the segmented X-axis add-reduces (f32 out).
Elementwise ops are batched across 2 nodes (v-stage) / 8 nodes (x2-stage)
to amortize per-instruction overheads.

Sharding: data-parallel over nodes b across 8 cores (128 nodes/core).
"""

import sys

if "/opt/trn_rl_repo" not in sys.path:
    sys.path.insert(0, "/opt/trn_rl_repo")

import numpy as np
import ml_dtypes

import concourse.bass as bass
import concourse.mybir as mybir
import concourse.tile as tile
from concourse.masks import make_identity

dt = mybir.dt

B, C, ELL, EQ, E = 1024, 128, 16, 3, 10
P3, P2, P1 = 23, 5, 1
N_CORES = 8
BS = B // N_CORES          # nodes per core (128)
WXV = EQ * ELL * ELL       # 768
WX2 = EQ * ELL             # 48
KCH = (128, 128, 112 + P2) # K chunks (chunk2: 112 U3-rows + 5 U2-rows)
CH = 16                    # nodes per DMA chunk
NCH = BS // CH             # chunks per core (8)
FC = CH * C                # chunk free width (2048)
HV = WXV // 2              # matmul N half (384)

_f32 = dt.float32
_bf16 = dt.bfloat16
_bf = ml_dtypes.bfloat16
_mult = mybir.AluOpType.mult
_add = mybir.AluOpType.add
_AX = mybir.AxisListType.X


def _build_program():
    nc = bass.Bass("TRN2", target_bir_lowering=False, debug=False)

    trep_d = nc.dram_tensor("trep", [3, 128, BS, C], _bf16, kind="ExternalInput")
    xf_d = nc.dram_tensor("xf", [C, BS, ELL], _bf16, kind="ExternalInput")
    q_d = nc.dram_tensor("q", [BS, C * EQ], _bf16, kind="ExternalInput")
    u3cat_d = nc.dram_tensor("u3cat", [3, 128, WXV], _bf16, kind="ExternalInput")
    out_d = nc.dram_tensor("out", [BS, C * EQ], _f32, kind="ExternalOutput")

    with tile.TileContext(nc) as tc:
        with tc.tile_pool(name="const", bufs=1) as cpool:
            u3sb = cpool.tile([128, 3, WXV], _bf16)
            nc.sync.dma_start(out=u3sb[:], in_=u3cat_d[:].rearrange("j p f -> p j f"))
            xfsb = cpool.tile([C, BS, ELL], _bf16)
            nc.sync.dma_start(out=xfsb[:], in_=xf_d[:])
            qsb = cpool.tile([BS, C * EQ], _bf16)
            nc.sync.dma_start(out=qsb[:], in_=q_d[:])
            outsb = cpool.tile([C, BS * EQ], _f32)    # [c, (b, w)] staging

            with tc.tile_pool(name="work", bufs=2) as pool, \
                 tc.tile_pool(name="pwork", bufs=3) as ppool, \
                 tc.tile_pool(name="owork", bufs=2) as opool, \
                 tc.tile_pool(name="ps1", bufs=3, space="PSUM") as psb:
                for ci in range(NCH):
                    bsl = slice(ci * CH, (ci + 1) * CH)
                    tch = pool.tile([128, 3, FC], _bf16, tag="tch")
                    nc.sync.dma_start(
                        out=tch[:], in_=trep_d[:, :, bsl].rearrange("j p b c -> p j b c")
                    )
                    for oi in range(CH // 8):      # 8-node blocks
                        ob = ci * CH + oi * 8
                        out2b = opool.tile([C, 8, WX2], _f32, tag="out2b")
                        for pi in range(4):        # node pairs
                            np0 = ob + 2 * pi
                            o1b = ppool.tile([C, 2, WXV], _bf16, tag="o1b")
                            for u in range(2):     # nodes in pair
                                n = np0 + u
                                nsl = slice((n - ci * CH) * C, (n - ci * CH + 1) * C)
                                # padded PSUM tile: each N=384 half in its own bank
                                ps1 = psb.tile([C, 2, 512], _f32, tag="ps1")
                                for h in range(2):
                                    for j in range(3):
                                        nc.tensor.matmul(
                                            ps1[:, h, :HV],
                                            tch[: KCH[j], j, nsl],
                                            u3sb[: KCH[j], j, h * HV : (h + 1) * HV],
                                            start=(j == 0),
                                            stop=(j == 2),
                                        )
                                nc.scalar.copy(
                                    o1b[:, u, :].rearrange("p (h f) -> p h f", h=2),
                                    ps1[:, :, :HV],
                                )
                            # v-contraction for the pair
                            xv2 = xfsb[:, np0 : np0 + 2, None, :]
                            m1 = ppool.tile([C, 2, WX2, ELL], _bf16, tag="m1")
                            nc.gpsimd.tensor_mul(
                                m1[:],
                                o1b[:].rearrange("p u (a v) -> p u a v", v=ELL),
                                xv2.to_broadcast([C, 2, WX2, ELL]),
                            )
                            nc.vector.tensor_reduce(
                                out2b[:, 2 * pi : 2 * pi + 2, :], m1[:], _AX, _add
                            )
                        # x2-contraction for the 8-node block
                        xv8 = xfsb[:, ob : ob + 8, None, :]
                        m2 = opool.tile([C, 8, EQ, ELL], _bf16, tag="m2")
                        nc.gpsimd.tensor_mul(
                            m2[:],
                            out2b[:].rearrange("p u (w i) -> p u w i", i=ELL),
                            xv8.to_broadcast([C, 8, EQ, ELL]),
                        )
                        nc.vector.tensor_reduce(
                            outsb[:, ob * EQ : (ob + 8) * EQ], m2[:], _AX, _add
                        )

            # -------- end phase: transpose [c,(b,w)] -> [b,(c,w)], + q --------
            with tc.tile_pool(name="fin", bufs=2) as fpool, \
                 tc.tile_pool(name="ps_fin", bufs=2, space="PSUM") as psf:
                ident128 = cpool.tile([128, 128], _f32)
                make_identity(nc, ident128[:])

                finsb = fpool.tile([BS, C * EQ], _f32, tag="finsb")
                outsb_r = outsb[:].rearrange("c (b w) -> c b w", w=EQ)
                finsb_r = finsb[:].rearrange("b (c w) -> b c w", w=EQ)
                qsb_r = qsb[:].rearrange("b (c w) -> b c w", w=EQ)
                for w in range(EQ):
                    fin_ps = psf.tile([BS, C], _f32, tag="fin")
                    nc.tensor.transpose(fin_ps[:], outsb_r[:, :, w], ident128[:])
                    nc.vector.tensor_add(finsb_r[:, :, w], fin_ps[:], qsb_r[:, :, w])

                nc.sync.dma_start(out=out_d[:], in_=finsb[:])

    import bass_rust
    bass_rust.move_matmul_waits_to_ldweights(nc.m)
    bass_rust.generate_event_semaphores(nc)
    return nc


def _host_prep(x, y, U3, U2, U1, w_max, w2, w1):
    x = np.ascontiguousarray(x, dtype=np.float32)
    elem = np.argmax(y, axis=1)

    wn3 = w_max[elem]                       # [B, 23, C]
    wn1 = w1[elem][:, 0, :]                 # [B, C]

    # trep[j, p, b, c] = x[b, c, i(p)] * wn3[b, 8j + p//16, c]; chunk2 rows
    # 112:117 = wn2 (folded U2 contraction operand)
    trep = np.zeros((B, 3, 128, C), dtype=np.float32)
    wn3r = np.repeat(wn3, ELL, axis=1)      # [B, 368, C]
    xtile = np.tile(x.transpose(0, 2, 1), (1, P3, 1))  # [B, 368, C]
    trep.reshape(B, 384, C)[:, :368, :] = wn3r * xtile
    trep[:, 2, 112 : 112 + P2, :] = w2[elem]
    trep = np.ascontiguousarray(trep.transpose(1, 2, 0, 3)).astype(_bf)  # [3,128,B,C]

    xf = np.ascontiguousarray(x.transpose(1, 0, 2)).astype(_bf)   # [C, B, ELL]

    # q[b, c, w] = wn1[b,c] * sum_x2 U1[w,x2]*x[b,c,x2]  (U1 path, host)
    q = wn1[:, :, None] * np.einsum("wi,bci->bcw", U1[:, :, 0], x)
    q = q.reshape(B, C * EQ).astype(_bf)

    # U3cat: [(k,i), (w, x2, v)] chunks of 128; chunk2 rows 112:117 = U2
    u3k = U3.transpose(4, 3, 0, 1, 2).reshape(ELL * P3, WXV)
    u2k = U2.transpose(3, 0, 1, 2).reshape(P2, WXV)
    u3cat = np.zeros((3, 128, WXV), dtype=np.float32)
    u3cat[0] = u3k[0:128]
    u3cat[1] = u3k[128:256]
    u3cat[2, 0:112] = u3k[256:368]
    u3cat[2, 112 : 112 + P2] = u2k
    u3cat = u3cat.astype(_bf)

    shared = {"u3cat": u3cat}

    def per_core(ci):
        s = slice(ci * BS, (ci + 1) * BS)
        m = {
            "trep": np.ascontiguousarray(trep[:, :, s]),
            "xf": np.ascontiguousarray(xf[:, s]),
            "q": np.ascontiguousarray(q[s]),
        }
        m.update(shared)
        return m

    return per_core


_PROGRAM_CACHE = {}


def kernel(**inputs) -> np.ndarray:
    from concourse.bass_utils import run_bass_kernel_spmd

    per_core = _host_prep(
        np.asarray(inputs["x"]), np.asarray(inputs["y"]),
        np.asarray(inputs["U3"]), np.asarray(inputs["U2"]),
        np.asarray(inputs["U1"]), np.asarray(inputs["w_max"]),
        np.asarray(inputs["w2"]), np.asarray(inputs["w1"]),
    )

    if "nc" not in _PROGRAM_CACHE:
        _PROGRAM_CACHE["nc"] = _build_program()
    nc = _PROGRAM_CACHE["nc"]

    in_maps = [per_core(ci) for ci in range(N_CORES)]
    res = run_bass_kernel_spmd(nc, in_maps, core_ids=list(range(N_CORES)))
    out = np.concatenate([r["out"] for r in res.results], axis=0)
    return out.astype(np.float32)


if __name__ == "__main__":
    from concourse.bass_interp import CoreSim

    rng = np.random.default_rng(0)
    x = rng.standard_normal((B, C, ELL)).astype(np.float32)
    elem = rng.integers(0, E, size=B)
    y = np.eye(E, dtype=np.float32)[elem]
    U3 = (rng.standard_normal((EQ, ELL, ELL, ELL, P3)) * 0.1).astype(np.float32)
    U2 = (rng.standard_normal((EQ, ELL, ELL, P2)) * 0.1).astype(np.float32)
    U1 = (rng.standard_normal((EQ, ELL, P1)) * 0.1).astype(np.float32)
    w_max = (rng.standard_normal((E, P3, C)) / P3).astype(np.float32)
    w2 = (rng.standard_normal((E, P2, C)) / P2).astype(np.float32)
    w1 = (rng.standard_normal((E, P1, C)) / P1).astype(np.float32)

    per_core = _host_prep(x, y, U3, U2, U1, w_max, w2, w1)
    nc = _build_program()
    sim = CoreSim(nc)
    m = per_core(0)
    for k, v in m.items():
        sim.tensor(k)[:] = v
    sim.simulate(check_with_hw=False, trace_hw=False)
    got = np.array(sim.tensor("out"))

    def ref_np(x, y, U3, U2, U1, w_max, w2, w1):
        wn3 = np.einsum("be,ekc->bkc", y, w_max)
        t = np.einsum("bkc,bci->bcik", wn3, x)
        out = np.einsum("wxvik,bcik->bcwxv", U3, t)
        wn2 = np.einsum("be,ekc->bkc", y, w2)
        c2 = np.einsum("wxvk,bkc->bcwxv", U2, wn2) + out
        out = np.einsum("bcwxi,bci->bcwx", c2, x)
        wn1 = np.einsum("be,ekc->bkc", y, w1)
        c1 = np.einsum("wxk,bkc->bcwx", U1, wn1) + out
        out = np.einsum("bcwi,bci->bcw", c1, x)
        return out.reshape(out.shape[0], -1)

    want = ref_np(x[:BS], y[:BS], U3, U2, U1, w_max, w2, w1)
    err = np.abs(got - want).max() / (np.abs(want).max() + 1e-30)
    print(f"CoreSim vs numpy rel err: {err:.3e}")
    assert err < 2e-2, "FAIL"
    print("SIM PASS")
